# revision 13
# baseline (speedup 1.0000x reference)
"""AttentionBlock (GroupNorm + single-head self-attention + residual) on 8 TRN2
NeuronCores, data-parallel over the batch dim (B=8, one batch element per core).

Per-core computation (C=512 channels, N=H*W=4096 pixels):
  stats: per-group mean/var of x -> per-channel affine a, b  (one [128,8] matmul
         against a group-indicator matrix for the cross-partition sums)
  h   = a*x + b        (groupnorm applied in the fp32->bf16 cast, one ACT pass)
  q/k = W h + bias     (bf16 matmuls)
  vT  = h^T Wv^T + bias broadcast   ([pixel, channel] layout)
  S^T = k^T q          (scores computed transposed: [k_pix, q_pix])
  P^T = exp(S^T / sqrt(C))   (no max-subtraction; scores are O(1) by construction)
  O   = v P^T          (PSUM accumulation over k tiles)
  denominator: DVE tile-accumulation of P^T + ones-matmul cross-partition sum;
  softmax normalization deferred past the output projection:
  out = x + bo + (Wo @ O) * broadcast(1/denom)
"""

import numpy as np
import ml_dtypes
from contextlib import ExitStack

import concourse.bass as bass
import concourse.tile as tile
from concourse import bacc, mybir
from concourse.bass_utils import run_bass_kernel_spmd

C = 512
GROUPS = 32
EPS = 1e-6
CT = C // 128          # 4 channel tiles of 128
CHUNK = 512            # q-chunk width (one PSUM bank of fp32)
F32 = mybir.dt.float32
BF16 = mybir.dt.bfloat16
FP8 = mybir.dt.float8e4
DR = mybir.MatmulPerfMode.DoubleRow
AF = mybir.ActivationFunctionType
ALU = mybir.AluOpType
AX = mybir.AxisListType

GPC = C // GROUPS      # channels per group = 16
GPT = 128 // GPC       # groups per channel-tile = 8


def build_nc(n_pix=4096, repeat=1):
    """repeat>1 wraps the whole body in a hardware loop — used only for timing
    (amortizes the ~80ms per-call axon dispatch overhead over R executions)."""
    nt = n_pix // 128          # number of 128-wide pixel tiles (k tiles)
    nchunk = n_pix // CHUNK    # number of q chunks
    inv_cnt = 1.0 / (GPC * n_pix)
    scale_s = 1.0 / float(np.sqrt(C))

    nc = bacc.Bacc(trn_type="TRN2", target_bir_lowering=False, debug=False)

    xd = nc.declare_dram_parameter("x", [C, n_pix], F32, isOutput=False)
    wqd = nc.declare_dram_parameter("wqT_bf", [C, C], BF16, isOutput=False)
    wkd = nc.declare_dram_parameter("wkT_bf", [C, C], BF16, isOutput=False)
    wvd = nc.declare_dram_parameter("wvT_bf", [C, C], BF16, isOutput=False)
    wod = nc.declare_dram_parameter("woT_bf", [C, C], BF16, isOutput=False)
    # per-channel vectors packed [128, CT]: column ct = channels ct*128..+128
    gsd = nc.declare_dram_parameter("gn_scale", [128, CT], F32, isOutput=False)
    gbd = nc.declare_dram_parameter("gn_bias", [128, CT], F32, isOutput=False)
    bqd = nc.declare_dram_parameter("bq", [128, CT], F32, isOutput=False)
    bkd = nc.declare_dram_parameter("bk", [128, CT], F32, isOutput=False)
    bvd = nc.declare_dram_parameter("bv", [128, CT], F32, isOutput=False)
    bod = nc.declare_dram_parameter("bo", [128, CT], F32, isOutput=False)
    outd = nc.declare_dram_parameter("out", [C, n_pix], F32, isOutput=True)

    gmat_np = np.zeros((128, GPT), np.float32)
    for p in range(128):
        gmat_np[p, p // GPC] = 1.0
    gmat_d = nc.inline_tensor(gmat_np, name="gmat")
    gmat_t_d = nc.inline_tensor(np.ascontiguousarray(gmat_np.T), name="gmat_t")
    ident_d = nc.inline_tensor(np.eye(128, dtype=np.float32), name="ident")
    ones_col_d = nc.inline_tensor(np.ones((128, 1), np.float32), name="ones_col")
    ones_col_b_d = nc.inline_tensor(
        np.ones((128, 1), ml_dtypes.bfloat16), name="ones_col_b")
    ones_row_f_d = nc.inline_tensor(np.ones((1, 128), np.float32), name="ones_row_f")
    ones_row_b_d = nc.inline_tensor(
        np.ones((1, 128), ml_dtypes.bfloat16), name="ones_row_b"
    )

    with tile.TileContext(nc) as tc, ExitStack() as ctx:
        cp = ctx.enter_context(tc.tile_pool(name="consts", bufs=1))
        res = ctx.enter_context(tc.tile_pool(name="res", bufs=1))
        xload = ctx.enter_context(tc.tile_pool(name="xload", bufs=6))
        hp = ctx.enter_context(tc.tile_pool(name="hp", bufs=8))
        scr = ctx.enter_context(tc.tile_pool(name="scr", bufs=2))
        qp = ctx.enter_context(tc.tile_pool(name="qp", bufs=8))
        ptp = ctx.enter_context(tc.tile_pool(name="ptp", bufs=4))
        accp = ctx.enter_context(tc.tile_pool(name="accp", bufs=2))
        rbp = ctx.enter_context(tc.tile_pool(name="rbp", bufs=2))
        oup = ctx.enter_context(tc.tile_pool(name="oup", bufs=8))
        ep = ctx.enter_context(tc.tile_pool(name="ep", bufs=3))
        psA = ctx.enter_context(tc.tile_pool(name="psA", bufs=2, space="PSUM"))
        psS = ctx.enter_context(tc.tile_pool(name="psS", bufs=2, space="PSUM"))
        psO = ctx.enter_context(tc.tile_pool(name="psO", bufs=4, space="PSUM"))

        if repeat > 1:
            loop_cm = tc.For_i(0, repeat)
            loop_cm.__enter__()

        # ---- stats-critical vectors first: the x DMAs must head the ring ----
        def load_vec(dram, label):
            t = cp.tile([128, CT], F32, name=label, tag=label)
            nc.sync.dma_start(t[:], dram.ap())
            return t

        gs_all = load_vec(gsd, "gs_all")
        gb_all = load_vec(gbd, "gb_all")
        gmat = cp.tile([128, GPT], F32, name="gmat_sb", tag="gmat")
        nc.sync.dma_start(gmat[:], gmat_d.ap())
        gmat_t = cp.tile([GPT, 128], F32, name="gmatT_sb", tag="gmatT")
        nc.sync.dma_start(gmat_t[:], gmat_t_d.ap())

        def load_w(dram, label):
            ws = []
            for ct in range(CT):
                t = res.tile([128, C], BF16, name=f"{label}{ct}", tag=f"{label}{ct}")
                nc.sync.dma_start(t[:], dram.ap()[ct * 128:(ct + 1) * 128, :])
                ws.append(t)
            return ws

        # ---- resident tensors ----
        x_bf = [res.tile([128, n_pix], BF16, name=f"x_bf{ct}", tag=f"x_bf{ct}")
                for ct in range(CT)]
        k2 = [res.tile([128, 2, n_pix], FP8, name=f"k2_{p}", tag=f"k2_{p}")
              for p in range(CT // 2)]
        vT2 = [res.tile([128, 2, C], FP8, name=f"vT2_{i}", tag=f"vT2_{i}")
               for i in range(nt // 2)]

        # ---- phase 1: load x, per-group stats, cast x->bf16 ----
        s_cols = [cp.tile([128, nchunk], F32, name=f"s_cols{ct}", tag=f"s_cols{ct}")
                  for ct in range(CT)]
        ss_cols = [cp.tile([128, nchunk], F32, name=f"ss_cols{ct}", tag=f"ss_cols{ct}")
                   for ct in range(CT)]
        for ct in range(CT):
            rows = slice(ct * 128, (ct + 1) * 128)
            for j in range(nchunk):
                cols = slice(j * CHUNK, (j + 1) * CHUNK)
                xs = xload.tile([128, CHUNK], F32, name=f"xs{ct}_{j}", tag="xs")
                nc.sync.dma_start(xs[:], xd.ap()[rows, cols])
                nc.vector.reduce_sum(s_cols[ct][:, j:j + 1], xs[:], axis=AX.X)
                sq = scr.tile([128, CHUNK], F32, name=f"sq{ct}_{j}", tag="sq")
                nc.scalar.activation(sq[:], xs[:], AF.Square,
                                     accum_out=ss_cols[ct][:, j:j + 1])
                nc.vector.tensor_copy(x_bf[ct][:, cols], xs[:])

        # remaining constants/vectors (not stats-critical)
        ident = cp.tile([128, 128], F32, name="ident_sb", tag="ident")
        nc.sync.dma_start(ident[:], ident_d.ap())
        ones_col = cp.tile([128, 1], F32, name="ones_col_sb", tag="ones_col")
        nc.sync.dma_start(ones_col[:], ones_col_d.ap())
        ones_col_b = cp.tile([128, 1], BF16, name="ones_col_b_sb", tag="ones_col_b")
        nc.sync.dma_start(ones_col_b[:], ones_col_b_d.ap())
        ones_row_f = cp.tile([1, 128], F32, name="ones_row_f_sb", tag="ones_row_f")
        nc.sync.dma_start(ones_row_f[:], ones_row_f_d.ap())
        ones_row_b = cp.tile([1, 128], BF16, name="ones_row_b_sb", tag="ones_row_b")
        nc.sync.dma_start(ones_row_b[:], ones_row_b_d.ap())
        bq_all = load_vec(bqd, "bq_all")
        bk_all = load_vec(bkd, "bk_all")
        bv_all = load_vec(bvd, "bv_all")
        bo_all = load_vec(bod, "bo_all")

        # weights loaded after x so the stats-critical x DMAs go first on the ring
        wk_bf = load_w(wkd, "wk")
        wv_bf = load_w(wvd, "wv")
        wq_bf = load_w(wqd, "wq")
        wo_bf = load_w(wod, "wo")

        # v-bias broadcast tile [128, C] (free-dim bias for the vT layout);
        # no dependency on stats, runs at t=0.
        bvrow = cp.tile([1, C], F32, name="bvrow", tag="bvrow")
        for ct in range(CT):
            ptr = psA.tile([1, 128], F32, name=f"ptr{ct}", tag="pa")
            nc.tensor.transpose(ptr[:], bv_all[:, ct:ct + 1], ident[:])
            nc.scalar.copy(bvrow[0:1, ct * 128:(ct + 1) * 128], ptr[:])
        pbb = psA.tile([128, C], F32, name="pbb", tag="pa")
        nc.tensor.matmul(pbb[:], lhsT=ones_row_f[:], rhs=bvrow[:],
                         start=True, stop=True)
        bvb = res.tile([128, C], F32, name="bvb", tag="bvb")
        nc.scalar.copy(bvb[:], pbb[:])

        stats_all = cp.tile([128, 2 * CT], F32, name="stats_all", tag="stats_all")
        for ct in range(CT):
            nc.vector.reduce_sum(stats_all[:, 2 * ct:2 * ct + 1], s_cols[ct][:],
                                 axis=AX.X)
            nc.vector.reduce_sum(stats_all[:, 2 * ct + 1:2 * ct + 2], ss_cols[ct][:],
                                 axis=AX.X)

        # one matmul for all cross-partition group sums: [128, 8] -> [8, 8]
        pg = psA.tile([GPT, 2 * CT], F32, name="pg", tag="pa")
        nc.tensor.matmul(pg[:], lhsT=gmat[:], rhs=stats_all[:], start=True, stop=True)
        gsb = cp.tile([GPT, 2 * CT], F32, name="gsb", tag="gsb")
        nc.scalar.copy(gsb[:], pg[:])

        mu44 = cp.tile([GPT, CT], F32, name="mu44", tag="mu44")
        ex2 = cp.tile([GPT, CT], F32, name="ex2", tag="ex2")
        musq = cp.tile([GPT, CT], F32, name="musq", tag="musq")
        var44 = cp.tile([GPT, CT], F32, name="var44", tag="var44")
        vare = cp.tile([GPT, CT], F32, name="vare", tag="vare")
        std44 = cp.tile([GPT, CT], F32, name="std44", tag="std44")
        rstd44 = cp.tile([GPT, CT], F32, name="rstd44", tag="rstd44")
        mr = cp.tile([GPT, 2 * CT], F32, name="mr", tag="mr")
        nc.scalar.mul(mu44[:], gsb[0:GPT, 0:2 * CT:2], inv_cnt)
        nc.scalar.mul(ex2[:], gsb[0:GPT, 1:2 * CT:2], inv_cnt)
        nc.vector.tensor_mul(musq[:], mu44[:], mu44[:])
        nc.vector.tensor_sub(var44[:], ex2[:], musq[:])
        nc.vector.tensor_scalar_add(vare[:], var44[:], EPS)
        nc.scalar.activation(std44[:], vare[:], AF.Sqrt)
        nc.vector.reciprocal(rstd44[:], std44[:])
        nc.vector.tensor_copy(mr[0:GPT, 0:2 * CT:2], mu44[:])
        nc.vector.tensor_copy(mr[0:GPT, 1:2 * CT:2], rstd44[:])

        # broadcast group mu/rstd back to channels: [8, 8] -> [128, 8]
        pmc = psA.tile([128, 2 * CT], F32, name="pmc", tag="pa")
        nc.tensor.matmul(pmc[:], lhsT=gmat_t[:], rhs=mr[:], start=True, stop=True)
        mcall = cp.tile([128, 2 * CT], F32, name="mcall", tag="mcall")
        nc.scalar.copy(mcall[:], pmc[:])
        a_all = cp.tile([128, CT], F32, name="a_all", tag="a_all")
        nc.vector.tensor_mul(a_all[:], mcall[:, 1:2 * CT:2], gs_all[:])
        btmp = cp.tile([128, CT], F32, name="btmp", tag="btmp")
        nc.vector.tensor_mul(btmp[:], mcall[:, 0:2 * CT:2], a_all[:])
        b_all = cp.tile([128, CT], F32, name="b_all", tag="b_all")
        nc.vector.tensor_sub(b_all[:], gb_all[:], btmp[:])

        # ---- phase 2: stream h = a*x_bf + b per chunk; k and vT projections ----
        def h_chunk(j, uid):
            """Cast one 512-wide chunk of h for all 4 channel tiles (2 on ACT,
            2 on DVE to split the latency)."""
            cols = slice(j * CHUNK, (j + 1) * CHUNK)
            hs = []
            for ct in range(CT):
                ht = hp.tile([128, CHUNK], BF16, name=f"h{uid}_{j}_{ct}", tag="h")
                if ct % 2 == 0:
                    nc.scalar.activation(ht[:], x_bf[ct][:, cols], AF.Identity,
                                         scale=a_all[:, ct:ct + 1],
                                         bias=b_all[:, ct:ct + 1])
                else:
                    nc.vector.tensor_scalar(ht[:], x_bf[ct][:, cols],
                                            a_all[:, ct:ct + 1],
                                            b_all[:, ct:ct + 1],
                                            op0=ALU.mult, op1=ALU.add)
                hs.append(ht)
            return hs

        for j in range(nchunk):
            cols = slice(j * CHUNK, (j + 1) * CHUNK)
            hs = h_chunk(j, "p2")
            for ct in range(CT):
                pk = psA.tile([128, CHUNK], F32, name=f"pk{ct}_{j}", tag="pa")
                for cpt in range(CT):
                    nc.tensor.matmul(pk[:],
                                     lhsT=wk_bf[cpt][:, ct * 128:(ct + 1) * 128],
                                     rhs=hs[cpt][:],
                                     start=(cpt == 0), stop=(cpt == CT - 1))
                kdst = k2[ct // 2][:, ct % 2, cols]
                if ct % 2 == 0:
                    nc.scalar.activation(kdst, pk[:], AF.Identity,
                                         bias=bk_all[:, ct:ct + 1])
                else:
                    nc.vector.tensor_scalar_add(kdst, pk[:],
                                                bk_all[:, ct:ct + 1])
            for i in range(4 * j, 4 * j + 4):
                off = (i - 4 * j) * 128
                pv = psA.tile([128, C], F32, name=f"pv{i}", tag="pa")
                for cpt in range(CT):
                    nc.tensor.matmul(pv[:],
                                     lhsT=hs[cpt][:, off:off + 128],
                                     rhs=wv_bf[cpt][:],
                                     start=(cpt == 0), stop=(cpt == CT - 1))
                nc.vector.tensor_add(vT2[i // 2][:, i % 2, :], pv[:], bvb[:])

        # ---- phase 3: attention, one q-chunk at a time; q-projection of the
        # next chunk is emitted inside the current chunk's tail so the PE never
        # waits on the serial softmax-denominator chain. ----
        def q_proj(ch):
            hs = h_chunk(ch, "q")
            qs = [qp.tile([128, 2, CHUNK], FP8, name=f"qs{ch}_{p}", tag="qs")
                  for p in range(CT // 2)]
            for ct in range(CT):
                pq = psA.tile([128, CHUNK], F32, name=f"pq{ch}_{ct}", tag="pa")
                for cpt in range(CT):
                    nc.tensor.matmul(pq[:],
                                     lhsT=wq_bf[cpt][:, ct * 128:(ct + 1) * 128],
                                     rhs=hs[cpt][:],
                                     start=(cpt == 0), stop=(cpt == CT - 1))
                qdst = qs[ct // 2][:, ct % 2, :]
                if ct % 2 == 0:
                    nc.scalar.activation(qdst, pq[:], AF.Identity,
                                         bias=bq_all[:, ct:ct + 1])
                else:
                    nc.vector.tensor_scalar_add(qdst, pq[:],
                                                bq_all[:, ct:ct + 1])
            return qs

        qs = q_proj(0)
        half = nt // 2
        for ch in range(nchunk):
            cols = slice(ch * CHUNK, (ch + 1) * CHUNK)
            po = [psO.tile([128, CHUNK], F32, name=f"po{ch}_{ct}", tag="po")
                  for ct in range(CT)]
            acc_a = accp.tile([128, CHUNK], BF16, name=f"acca{ch}", tag="acca")
            acc_b = accp.tile([128, CHUNK], BF16, name=f"accb{ch}", tag="accb")
            npair = nt // 2
            pts = [None] * npair

            def o_pair(pp):
                for ct in range(CT):
                    nc.tensor.matmul(po[ct][:],
                                     lhsT=vT2[pp][:, :, ct * 128:(ct + 1) * 128],
                                     rhs=pts[pp][:],
                                     start=(pp == 0), stop=(pp == npair - 1),
                                     perf_mode=DR)

            for kt in range(nt):
                ps = psS.tile([128, CHUNK], F32, name=f"ps{ch}_{kt}", tag="ps")
                for p in range(CT // 2):
                    nc.tensor.matmul(ps[:],
                                     lhsT=k2[p][:, :, kt * 128:(kt + 1) * 128],
                                     rhs=qs[p][:],
                                     start=(p == 0), stop=(p == CT // 2 - 1),
                                     perf_mode=DR)
                if kt % 2 == 0:
                    pts[kt // 2] = ptp.tile([128, 2, CHUNK], FP8,
                                            name=f"pt{ch}_{kt}", tag="pt")
                pt_half = pts[kt // 2][:, kt % 2, :]
                nc.scalar.activation(pt_half, ps[:], AF.Exp, scale=scale_s)
                # softmax denominator, accumulated in two halves so the
                # cross-partition sum can start at half time
                acc = acc_a if kt < half else acc_b
                if kt == 0 or kt == half:
                    nc.vector.tensor_copy(acc[:], pt_half)
                else:
                    nc.vector.tensor_add(acc[:], acc[:], pt_half)
                # O matmuls lag one completed pair (keeps PE off the ACT path)
                if kt % 2 == 1 and kt >= 3:
                    o_pair((kt - 1) // 2 - 1)
            o_pair(npair - 1)

            # next chunk's q projection fills the PE while the denominator
            # chain (matmul -> reciprocal -> broadcast-matmul) resolves
            if ch + 1 < nchunk:
                qs_next = q_proj(ch + 1)

            pd = psA.tile([1, CHUNK], F32, name=f"pd{ch}", tag="pa")
            nc.tensor.matmul(pd[:], lhsT=ones_col_b[:], rhs=acc_a[:],
                             start=True, stop=False)
            nc.tensor.matmul(pd[:], lhsT=ones_col_b[:], rhs=acc_b[:],
                             start=False, stop=True)
            r32 = rbp.tile([1, CHUNK], F32, name=f"r32_{ch}", tag="r32")
            nc.vector.reciprocal(r32[:], pd[:])
            rbf = rbp.tile([1, CHUNK], BF16, name=f"rbf{ch}", tag="rbf")
            nc.vector.tensor_copy(rbf[:], r32[:])
            prb = psA.tile([128, CHUNK], F32, name=f"prb{ch}", tag="pa")
            nc.tensor.matmul(prb[:], lhsT=ones_row_b[:], rhs=rbf[:],
                             start=True, stop=True)
            rb = rbp.tile([128, CHUNK], F32, name=f"rb{ch}", tag="rb")
            nc.scalar.copy(rb[:], prb[:])

            # unnormalized O -> bf16 sbuf (split ACT/DVE to halve the latency)
            ou = []
            for ct in range(CT):
                t = oup.tile([128, CHUNK], BF16, name=f"ou{ch}_{ct}", tag="ou")
                if ct % 2 == 0:
                    nc.scalar.copy(t[:], po[ct][:])
                else:
                    nc.vector.tensor_copy(t[:], po[ct][:])
                ou.append(t)

            # output projection + deferred normalization + residual + bo
            for oct in range(CT):
                pz = psA.tile([128, CHUNK], F32, name=f"pz{ch}_{oct}", tag="pa")
                for ct in range(CT):
                    nc.tensor.matmul(pz[:],
                                     lhsT=wo_bf[ct][:, oct * 128:(oct + 1) * 128],
                                     rhs=ou[ct][:],
                                     start=(ct == 0), stop=(ct == CT - 1))
                xr = ep.tile([128, CHUNK], F32, name=f"xr{ch}_{oct}", tag="xr")
                nc.sync.dma_start(xr[:], xd.ap()[oct * 128:(oct + 1) * 128, cols])
                t1 = ep.tile([128, CHUNK], F32, name=f"t1_{ch}_{oct}", tag="t1")
                nc.vector.tensor_mul(t1[:], pz[:], rb[:])
                osb = ep.tile([128, CHUNK], F32, name=f"osb{ch}_{oct}", tag="osb")
                nc.vector.scalar_tensor_tensor(osb[:], t1[:], bo_all[:, oct:oct + 1],
                                               xr[:], op0=ALU.add, op1=ALU.add)
                nc.sync.dma_start(outd.ap()[oct * 128:(oct + 1) * 128, cols], osb[:])

            if ch + 1 < nchunk:
                qs = qs_next

        if repeat > 1:
            loop_cm.__exit__(None, None, None)

    nc.compile()
    return nc


_NC_CACHE = {}


def _get_nc(n_pix):
    if n_pix not in _NC_CACHE:
        _NC_CACHE[n_pix] = build_nc(n_pix)
    return _NC_CACHE[n_pix]


def make_in_maps(x, gn_scale, gn_bias, Wq, bq, Wk, bk, Wv, bv, Wo, bo):
    B, C_, H, W = x.shape
    n_pix = H * W

    def vec(v):
        return np.ascontiguousarray(
            np.asarray(v, np.float32).reshape(CT, 128).T)

    def wT_bf(w):
        return np.ascontiguousarray(
            np.asarray(w, np.float32).T.astype(ml_dtypes.bfloat16))

    base = {
        "wqT_bf": wT_bf(Wq),
        "wkT_bf": wT_bf(Wk),
        "wvT_bf": wT_bf(Wv),
        "woT_bf": wT_bf(Wo),
        "gn_scale": vec(gn_scale),
        "gn_bias": vec(gn_bias),
        "bq": vec(bq),
        "bk": vec(bk),
        "bv": vec(bv),
        "bo": vec(bo),
    }
    f32 = lambda v: np.ascontiguousarray(np.asarray(v, np.float32))
    return [dict(base, x=f32(np.asarray(x[b], np.float32).reshape(C_, n_pix)))
            for b in range(B)]


def kernel(x, gn_scale, gn_bias, Wq, bq, Wk, bk, Wv, bv, Wo, bo):
    x = np.asarray(x)
    B, C_, H, W = x.shape
    n_pix = H * W
    nc = _get_nc(n_pix)
    in_maps = make_in_maps(x, gn_scale, gn_bias, Wq, bq, Wk, bk, Wv, bv, Wo, bo)
    res = run_bass_kernel_spmd(nc, in_maps, core_ids=list(range(B)))
    out = np.stack([res.results[b]["out"] for b in range(B)])
    return out.reshape(B, C_, H, W).astype(np.float32)


# revision 21
# speedup vs baseline: 1.4036x; 1.4036x over previous
"""AttentionBlock (GroupNorm + single-head self-attention + residual) on 8 TRN2
NeuronCores, data-parallel over the batch dim (B=8, one batch element per core).

Per-core computation (C=512 channels, N=H*W=4096 pixels):
  stats: per-group mean/var of x -> per-channel affine a, b  (one [128,8] matmul
         against a group-indicator matrix for the cross-partition sums)
  h   = a*x + b        (groupnorm applied in the fp32->bf16 cast, one ACT pass)
  q/k = W h + bias     (bf16 matmuls)
  vT  = h^T Wv^T + bias broadcast   ([pixel, channel] layout)
  S^T = k^T q          (scores computed transposed: [k_pix, q_pix])
  P^T = exp(S^T / sqrt(C))   (no max-subtraction; scores are O(1) by construction)
  O   = v P^T          (PSUM accumulation over k tiles)
  denominator: DVE tile-accumulation of P^T + ones-matmul cross-partition sum;
  softmax normalization deferred past the output projection:
  out = x + bo + (Wo @ O) * broadcast(1/denom)
"""

import numpy as np
import ml_dtypes
from contextlib import ExitStack

import concourse.bass as bass
import concourse.tile as tile
from concourse import bacc, mybir
from concourse.bass_utils import run_bass_kernel_spmd

C = 512
GROUPS = 32
EPS = 1e-6
CT = C // 128          # 4 channel tiles of 128
CHUNK = 512            # q-chunk width (one PSUM bank of fp32)
F32 = mybir.dt.float32
BF16 = mybir.dt.bfloat16
FP8 = mybir.dt.float8e4
DR = mybir.MatmulPerfMode.DoubleRow
AF = mybir.ActivationFunctionType
ALU = mybir.AluOpType
AX = mybir.AxisListType

GPC = C // GROUPS      # channels per group = 16
GPT = 128 // GPC       # groups per channel-tile = 8


def build_nc(n_pix=4096, repeat=1):
    """repeat>1 wraps the whole body in a hardware loop — used only for timing
    (amortizes the ~80ms per-call axon dispatch overhead over R executions)."""
    nt = n_pix // 128          # number of 128-wide pixel tiles (k tiles)
    nchunk = n_pix // CHUNK    # number of q chunks
    inv_cnt = 1.0 / (GPC * n_pix)
    scale_s = 1.0 / float(np.sqrt(C))

    nc = bacc.Bacc(trn_type="TRN2", target_bir_lowering=False, debug=False)

    xd = nc.declare_dram_parameter("x", [C, n_pix], F32, isOutput=False)
    wqd = nc.declare_dram_parameter("wqT2", [CT // 2, 128, 2, C], FP8, isOutput=False)
    wkd = nc.declare_dram_parameter("wkT2", [CT // 2, 128, 2, C], FP8, isOutput=False)
    wvd = nc.declare_dram_parameter("wvT2", [CT // 2, 128, 2, C], FP8, isOutput=False)
    wod = nc.declare_dram_parameter("woT2", [CT // 2, 128, 2, C], FP8, isOutput=False)
    # per-channel vectors packed [128, CT]: column ct = channels ct*128..+128
    gsd = nc.declare_dram_parameter("gn_scale", [128, CT], F32, isOutput=False)
    gbd = nc.declare_dram_parameter("gn_bias", [128, CT], F32, isOutput=False)
    bqd = nc.declare_dram_parameter("bq", [128, CT], F32, isOutput=False)
    bkd = nc.declare_dram_parameter("bk", [128, CT], F32, isOutput=False)
    bod = nc.declare_dram_parameter("bo", [128, CT], F32, isOutput=False)
    outd = nc.declare_dram_parameter("out", [C, n_pix], F32, isOutput=True)

    gmat_np = np.zeros((128, GPT), np.float32)
    for p in range(128):
        gmat_np[p, p // GPC] = 1.0
    gmat_d = nc.inline_tensor(gmat_np, name="gmat")
    gmat_t_d = nc.inline_tensor(np.ascontiguousarray(gmat_np.T), name="gmat_t")
    ident_d = nc.inline_tensor(np.eye(128, dtype=np.float32), name="ident")
    ones_col_d = nc.inline_tensor(np.ones((128, 1), np.float32), name="ones_col")
    ones_col_b_d = nc.inline_tensor(
        np.ones((128, 1), ml_dtypes.bfloat16), name="ones_col_b")
    ones_row_f_d = nc.inline_tensor(np.ones((1, 128), np.float32), name="ones_row_f")
    ones_row_b_d = nc.inline_tensor(
        np.ones((1, 128), ml_dtypes.bfloat16), name="ones_row_b"
    )
    # pair-dim stride must be a multiple of 16 for DoubleRow ldweights
    ones_pair_d = nc.inline_tensor(
        np.ones((128, 2, 16), ml_dtypes.float8_e4m3), name="ones_pair")

    with tile.TileContext(nc) as tc, ExitStack() as ctx:
        cp = ctx.enter_context(tc.tile_pool(name="consts", bufs=1))
        res = ctx.enter_context(tc.tile_pool(name="res", bufs=1))
        xload = ctx.enter_context(tc.tile_pool(name="xload", bufs=6))
        hp = ctx.enter_context(tc.tile_pool(name="hp", bufs=8))
        scr = ctx.enter_context(tc.tile_pool(name="scr", bufs=2))
        qp = ctx.enter_context(tc.tile_pool(name="qp", bufs=8))
        ptp = ctx.enter_context(tc.tile_pool(name="ptp", bufs=4))
        accp = ctx.enter_context(tc.tile_pool(name="accp", bufs=2))
        rbp = ctx.enter_context(tc.tile_pool(name="rbp", bufs=2))
        oup = ctx.enter_context(tc.tile_pool(name="oup", bufs=8))
        ep = ctx.enter_context(tc.tile_pool(name="ep", bufs=3))
        psA = ctx.enter_context(tc.tile_pool(name="psA", bufs=2, space="PSUM"))
        psS = ctx.enter_context(tc.tile_pool(name="psS", bufs=2, space="PSUM"))
        psO = ctx.enter_context(tc.tile_pool(name="psO", bufs=4, space="PSUM"))

        if repeat > 1:
            loop_cm = tc.For_i(0, repeat)
            loop_cm.__enter__()

        # ---- stats-critical vectors first: the x DMAs must head the ring ----
        def load_vec(dram, label):
            t = cp.tile([128, CT], F32, name=label, tag=label)
            nc.sync.dma_start(t[:], dram.ap())
            return t

        gs_all = load_vec(gsd, "gs_all")
        gb_all = load_vec(gbd, "gb_all")
        gmat = cp.tile([128, GPT], F32, name="gmat_sb", tag="gmat")
        nc.sync.dma_start(gmat[:], gmat_d.ap())
        gmat_t = cp.tile([GPT, 128], F32, name="gmatT_sb", tag="gmatT")
        nc.sync.dma_start(gmat_t[:], gmat_t_d.ap())

        def load_w(dram, label):
            ws = []
            for p in range(CT // 2):
                t = res.tile([128, 2, C], FP8, name=f"{label}{p}", tag=f"{label}{p}")
                nc.sync.dma_start(t[:], dram.ap()[p])
                ws.append(t)
            return ws

        # ---- resident tensors ----
        x_bf = [res.tile([128, n_pix], BF16, name=f"x_bf{ct}", tag=f"x_bf{ct}")
                for ct in range(CT)]
        k2 = [res.tile([128, 2, n_pix], FP8, name=f"k2_{p}", tag=f"k2_{p}")
              for p in range(CT // 2)]
        vT2 = [res.tile([128, 2, C], FP8, name=f"vT2_{i}", tag=f"vT2_{i}")
               for i in range(nt // 2)]

        # ---- phase 1: load x, per-group stats, cast x->bf16 ----
        s_cols = [cp.tile([128, nchunk], F32, name=f"s_cols{ct}", tag=f"s_cols{ct}")
                  for ct in range(CT)]
        ss_cols = [cp.tile([128, nchunk], F32, name=f"ss_cols{ct}", tag=f"ss_cols{ct}")
                   for ct in range(CT)]
        for ct in range(CT):
            rows = slice(ct * 128, (ct + 1) * 128)
            for j in range(nchunk):
                cols = slice(j * CHUNK, (j + 1) * CHUNK)
                xs = xload.tile([128, CHUNK], F32, name=f"xs{ct}_{j}", tag="xs")
                nc.sync.dma_start(xs[:], xd.ap()[rows, cols])
                nc.vector.reduce_sum(s_cols[ct][:, j:j + 1], xs[:], axis=AX.X)
                sq = scr.tile([128, CHUNK], F32, name=f"sq{ct}_{j}", tag="sq")
                nc.scalar.activation(sq[:], xs[:], AF.Square,
                                     accum_out=ss_cols[ct][:, j:j + 1])
                nc.vector.tensor_copy(x_bf[ct][:, cols], xs[:])

        # remaining constants/vectors (not stats-critical)
        ones_col = cp.tile([128, 1], F32, name="ones_col_sb", tag="ones_col")
        nc.sync.dma_start(ones_col[:], ones_col_d.ap())
        ones_col_b = cp.tile([128, 1], BF16, name="ones_col_b_sb", tag="ones_col_b")
        nc.sync.dma_start(ones_col_b[:], ones_col_b_d.ap())
        ones_row_b = cp.tile([1, 128], BF16, name="ones_row_b_sb", tag="ones_row_b")
        nc.sync.dma_start(ones_row_b[:], ones_row_b_d.ap())
        ones_pair = cp.tile([128, 2, 16], FP8, name="ones_pair_sb", tag="ones_pair")
        nc.sync.dma_start(ones_pair[:], ones_pair_d.ap())
        bq_all = load_vec(bqd, "bq_all")
        bk_all = load_vec(bkd, "bk_all")
        bo_all = load_vec(bod, "bo_all")

        # weights loaded after x so the stats-critical x DMAs go first on the ring
        wk_bf = load_w(wkd, "wk")
        wv_bf = load_w(wvd, "wv")
        wq_bf = load_w(wqd, "wq")
        wo_bf = load_w(wod, "wo")

        stats_all = cp.tile([128, 2 * CT], F32, name="stats_all", tag="stats_all")
        for ct in range(CT):
            nc.vector.reduce_sum(stats_all[:, 2 * ct:2 * ct + 1], s_cols[ct][:],
                                 axis=AX.X)
            nc.vector.reduce_sum(stats_all[:, 2 * ct + 1:2 * ct + 2], ss_cols[ct][:],
                                 axis=AX.X)

        # one matmul for all cross-partition group sums: [128, 8] -> [8, 8]
        pg = psA.tile([GPT, 2 * CT], F32, name="pg", tag="pa")
        nc.tensor.matmul(pg[:], lhsT=gmat[:], rhs=stats_all[:], start=True, stop=True)
        gsb = cp.tile([GPT, 2 * CT], F32, name="gsb", tag="gsb")
        nc.scalar.copy(gsb[:], pg[:])

        mu44 = cp.tile([GPT, CT], F32, name="mu44", tag="mu44")
        ex2 = cp.tile([GPT, CT], F32, name="ex2", tag="ex2")
        musq = cp.tile([GPT, CT], F32, name="musq", tag="musq")
        var44 = cp.tile([GPT, CT], F32, name="var44", tag="var44")
        vare = cp.tile([GPT, CT], F32, name="vare", tag="vare")
        std44 = cp.tile([GPT, CT], F32, name="std44", tag="std44")
        rstd44 = cp.tile([GPT, CT], F32, name="rstd44", tag="rstd44")
        mr = cp.tile([GPT, 2 * CT], F32, name="mr", tag="mr")
        nc.scalar.mul(mu44[:], gsb[0:GPT, 0:2 * CT:2], inv_cnt)
        nc.scalar.mul(ex2[:], gsb[0:GPT, 1:2 * CT:2], inv_cnt)
        nc.vector.tensor_mul(musq[:], mu44[:], mu44[:])
        nc.vector.tensor_sub(var44[:], ex2[:], musq[:])
        nc.vector.tensor_scalar_add(vare[:], var44[:], EPS)
        nc.scalar.activation(std44[:], vare[:], AF.Sqrt)
        nc.vector.reciprocal(rstd44[:], std44[:])
        nc.vector.tensor_copy(mr[0:GPT, 0:2 * CT:2], mu44[:])
        nc.vector.tensor_copy(mr[0:GPT, 1:2 * CT:2], rstd44[:])

        # broadcast group mu/rstd back to channels: [8, 8] -> [128, 8]
        pmc = psA.tile([128, 2 * CT], F32, name="pmc", tag="pa")
        nc.tensor.matmul(pmc[:], lhsT=gmat_t[:], rhs=mr[:], start=True, stop=True)
        mcall = cp.tile([128, 2 * CT], F32, name="mcall", tag="mcall")
        nc.scalar.copy(mcall[:], pmc[:])
        a_all = cp.tile([128, CT], F32, name="a_all", tag="a_all")
        nc.vector.tensor_mul(a_all[:], mcall[:, 1:2 * CT:2], gs_all[:])
        btmp = cp.tile([128, CT], F32, name="btmp", tag="btmp")
        nc.vector.tensor_mul(btmp[:], mcall[:, 0:2 * CT:2], a_all[:])
        b_all = cp.tile([128, CT], F32, name="b_all", tag="b_all")
        nc.vector.tensor_sub(b_all[:], gb_all[:], btmp[:])

        # ---- phase 2: stream h = a*x_bf + b per chunk; k and vT projections ----
        def h_chunk(j, uid, dve_only=False):
            """Cast one 512-wide chunk of h (fp8 pair layout) for all 4 channel
            tiles. Phase 2 splits ACT/DVE; the attention phase keeps ACT free
            for exp and runs the casts on DVE."""
            cols = slice(j * CHUNK, (j + 1) * CHUNK)
            hs = [hp.tile([128, 2, CHUNK], FP8, name=f"h{uid}_{j}_{p}", tag="h")
                  for p in range(CT // 2)]
            for ct in range(CT):
                hdst = hs[ct // 2][:, ct % 2, :]
                if ct % 2 == 0 and not dve_only:
                    nc.scalar.activation(hdst, x_bf[ct][:, cols], AF.Identity,
                                         scale=a_all[:, ct:ct + 1],
                                         bias=b_all[:, ct:ct + 1])
                else:
                    nc.vector.tensor_scalar(hdst, x_bf[ct][:, cols],
                                            a_all[:, ct:ct + 1],
                                            b_all[:, ct:ct + 1],
                                            op0=ALU.mult, op1=ALU.add)
            return hs

        for j in range(nchunk):
            cols = slice(j * CHUNK, (j + 1) * CHUNK)
            hs = h_chunk(j, "p2")  # fp8 pair tiles for this chunk
            for ct in range(CT):
                pk = psA.tile([128, CHUNK], F32, name=f"pk{ct}_{j}", tag="pa")
                for p in range(CT // 2):
                    nc.tensor.matmul(pk[:],
                                     lhsT=wk_bf[p][:, :, ct * 128:(ct + 1) * 128],
                                     rhs=hs[p][:],
                                     start=(p == 0), stop=(p == CT // 2 - 1),
                                     perf_mode=DR)
                kdst = k2[ct // 2][:, ct % 2, cols]
                nc.vector.tensor_scalar_add(kdst, pk[:], bk_all[:, ct:ct + 1])
            for i in range(4 * j, 4 * j + 4):
                off = (i - 4 * j) * 128
                pv = psA.tile([128, C], F32, name=f"pv{i}", tag="pa")
                for p in range(CT // 2):
                    nc.tensor.matmul(pv[:],
                                     lhsT=hs[p][:, :, off:off + 128],
                                     rhs=wv_bf[p][:],
                                     start=(p == 0), stop=(p == CT // 2 - 1),
                                     perf_mode=DR)
                vdst = vT2[i // 2][:, i % 2, :]
                if i % 2 == 0:
                    nc.scalar.copy(vdst, pv[:])
                else:
                    nc.vector.tensor_copy(vdst, pv[:])

        # ---- phase 3: attention, one q-chunk at a time; q-projection of the
        # next chunk is emitted inside the current chunk's tail so the PE never
        # waits on the serial softmax-denominator chain. ----
        def q_proj(ch):
            hs = h_chunk(ch, "q")
            qs = [qp.tile([128, 2, CHUNK], FP8, name=f"qs{ch}_{p}", tag="qs")
                  for p in range(CT // 2)]
            for ct in range(CT):
                pq = psA.tile([128, CHUNK], F32, name=f"pq{ch}_{ct}", tag="pa")
                for p in range(CT // 2):
                    nc.tensor.matmul(pq[:],
                                     lhsT=wq_bf[p][:, :, ct * 128:(ct + 1) * 128],
                                     rhs=hs[p][:],
                                     start=(p == 0), stop=(p == CT // 2 - 1),
                                     perf_mode=DR)
                qdst = qs[ct // 2][:, ct % 2, :]
                nc.vector.tensor_scalar_add(qdst, pq[:], bq_all[:, ct:ct + 1])
            return qs

        qs = q_proj(0)
        for ch in range(nchunk):
            cols = slice(ch * CHUNK, (ch + 1) * CHUNK)
            po = [psO.tile([128, CHUNK], F32, name=f"po{ch}_{ct}", tag="po")
                  for ct in range(CT)]
            pd = psA.tile([1, CHUNK], F32, name=f"pd{ch}", tag="pa")
            npair = nt // 2
            pts = [None] * npair

            def o_pair(pp):
                for ct in range(CT):
                    nc.tensor.matmul(po[ct][:],
                                     lhsT=vT2[pp][:, :, ct * 128:(ct + 1) * 128],
                                     rhs=pts[pp][:],
                                     start=(pp == 0), stop=(pp == npair - 1),
                                     perf_mode=DR)
                nc.tensor.matmul(pd[:], lhsT=ones_pair[:, :, 0:1],
                                 rhs=pts[pp][:],
                                 start=(pp == 0), stop=(pp == npair - 1),
                                 perf_mode=DR)

            for kt in range(nt):
                ps = psS.tile([128, CHUNK], F32, name=f"ps{ch}_{kt}", tag="ps")
                for p in range(CT // 2):
                    nc.tensor.matmul(ps[:],
                                     lhsT=k2[p][:, :, kt * 128:(kt + 1) * 128],
                                     rhs=qs[p][:],
                                     start=(p == 0), stop=(p == CT // 2 - 1),
                                     perf_mode=DR)
                if kt % 2 == 0:
                    pts[kt // 2] = ptp.tile([128, 2, CHUNK], FP8,
                                            name=f"pt{ch}_{kt}", tag="pt")
                pt_half = pts[kt // 2][:, kt % 2, :]
                nc.scalar.activation(pt_half, ps[:], AF.Exp, scale=scale_s)
                # O matmuls lag one completed pair (keeps PE off the ACT path)
                if kt % 2 == 1 and kt >= 3:
                    o_pair((kt - 1) // 2 - 1)
            o_pair(npair - 1)
            r32 = rbp.tile([1, CHUNK], F32, name=f"r32_{ch}", tag="r32")
            nc.vector.reciprocal(r32[:], pd[:])
            rbf = rbp.tile([1, CHUNK], BF16, name=f"rbf{ch}", tag="rbf")
            nc.vector.tensor_copy(rbf[:], r32[:])

            # next chunk's q projection fills the PE while the denominator
            # chain resolves
            if ch + 1 < nchunk:
                qs_next = q_proj(ch + 1)

            prb = psA.tile([128, CHUNK], F32, name=f"prb{ch}", tag="pa")
            nc.tensor.matmul(prb[:], lhsT=ones_row_b[:], rhs=rbf[:],
                             start=True, stop=True)
            rb = rbp.tile([128, CHUNK], F32, name=f"rb{ch}", tag="rb")
            nc.vector.tensor_copy(rb[:], prb[:])

            # unnormalized O -> fp8 pair sbuf (split ACT/DVE for latency)
            ou = [oup.tile([128, 2, CHUNK], FP8, name=f"ou{ch}_{p}", tag="ou")
                  for p in range(CT // 2)]
            for ct in range(CT):
                odst = ou[ct // 2][:, ct % 2, :]
                if ct % 2 == 0:
                    nc.scalar.copy(odst, po[ct][:])
                else:
                    nc.vector.tensor_copy(odst, po[ct][:])

            # output projection + deferred normalization + residual + bo
            for oct in range(CT):
                pz = psA.tile([128, CHUNK], F32, name=f"pz{ch}_{oct}", tag="pa")
                for p in range(CT // 2):
                    nc.tensor.matmul(pz[:],
                                     lhsT=wo_bf[p][:, :, oct * 128:(oct + 1) * 128],
                                     rhs=ou[p][:],
                                     start=(p == 0), stop=(p == CT // 2 - 1),
                                     perf_mode=DR)
                xr = ep.tile([128, CHUNK], F32, name=f"xr{ch}_{oct}", tag="xr")
                nc.sync.dma_start(xr[:], xd.ap()[oct * 128:(oct + 1) * 128, cols])
                t1 = ep.tile([128, CHUNK], F32, name=f"t1_{ch}_{oct}", tag="t1")
                nc.vector.tensor_mul(t1[:], pz[:], rb[:])
                osb = ep.tile([128, CHUNK], F32, name=f"osb{ch}_{oct}", tag="osb")
                nc.vector.scalar_tensor_tensor(osb[:], t1[:], bo_all[:, oct:oct + 1],
                                               xr[:], op0=ALU.add, op1=ALU.add)
                nc.sync.dma_start(outd.ap()[oct * 128:(oct + 1) * 128, cols], osb[:])

            if ch + 1 < nchunk:
                qs = qs_next

        if repeat > 1:
            loop_cm.__exit__(None, None, None)

    nc.compile()
    return nc


_NC_CACHE = {}


def _get_nc(n_pix):
    if n_pix not in _NC_CACHE:
        _NC_CACHE[n_pix] = build_nc(n_pix)
    return _NC_CACHE[n_pix]


def make_in_maps(x, gn_scale, gn_bias, Wq, bq, Wk, bk, Wv, bv, Wo, bo):
    B, C_, H, W = x.shape
    n_pix = H * W

    def vec(v):
        return np.ascontiguousarray(
            np.asarray(v, np.float32).reshape(CT, 128).T)

    def wT2(w):
        """wT [C, C] -> pair-packed [CT//2, 128, 2, C] fp8 (DoubleRow layout)."""
        wt = np.asarray(w, np.float32).T.reshape(CT // 2, 2, 128, C)
        return np.ascontiguousarray(
            wt.transpose(0, 2, 1, 3).astype(ml_dtypes.float8_e4m3))

    # v-bias folds into the output bias: softmax rows sum to 1, so
    # out = x + Wo @ (v_0 P^T / denom) + (bo + Wo @ bv)
    bo_eff = (np.asarray(bo, np.float64)
              + np.asarray(Wo, np.float64) @ np.asarray(bv, np.float64))
    base = {
        "wqT2": wT2(Wq),
        "wkT2": wT2(Wk),
        "wvT2": wT2(Wv),
        "woT2": wT2(Wo),
        "gn_scale": vec(gn_scale),
        "gn_bias": vec(gn_bias),
        "bq": vec(bq),
        "bk": vec(bk),
        "bo": vec(bo_eff),
    }
    f32 = lambda v: np.ascontiguousarray(np.asarray(v, np.float32))
    return [dict(base, x=f32(np.asarray(x[b], np.float32).reshape(C_, n_pix)))
            for b in range(B)]


def kernel(x, gn_scale, gn_bias, Wq, bq, Wk, bk, Wv, bv, Wo, bo):
    x = np.asarray(x)
    B, C_, H, W = x.shape
    n_pix = H * W
    nc = _get_nc(n_pix)
    in_maps = make_in_maps(x, gn_scale, gn_bias, Wq, bq, Wk, bk, Wv, bv, Wo, bo)
    res = run_bass_kernel_spmd(nc, in_maps, core_ids=list(range(B)))
    out = np.stack([res.results[b]["out"] for b in range(B)])
    return out.reshape(B, C_, H, W).astype(np.float32)
